# revision 7
# baseline (speedup 1.0000x reference)
"""Trainium2 Bass kernel for per-channel EMA (first-order linear recurrence).

y[:, :, t] = w*x[:, :, t] + (1-w)*y[:, :, t-1],  y[:, :, -1] := x[:, :, 0]

Sharding: data-parallel over batch across 8 NeuronCores (8 batches/core).
Per core, per batch: channels (128) on the partition dim, time (8192) on the
free dim; premultiply b = w*x, then one native TensorTensorScan (both on the
Vector engine) computes the full recurrence. Memory-bound at the HBM
roofline: 64MB/core traffic at ~360GB/s ≈ 178us; measured ~170-200us.
"""

from contextlib import ExitStack

import numpy as np

# Hardcoded problem shape (self-contained; do not read spec/reference).
B, C, T = 64, 128, 8192
N_CORES = 8
B_SHARD = B // N_CORES


def _build_bass(nb=B_SHARD, ch=C, t=T, t_chunk=None, reps=1):
    import concourse.tile as tile
    from concourse import bacc, mybir

    if t_chunk is None:
        t_chunk = t
    assert t % t_chunk == 0
    n_chunks = t // t_chunk

    f32 = mybir.dt.float32
    # Bacc (not raw Bass): its compile() runs generate_event_semaphores(),
    # which splits multi-sem waits to satisfy the 1-wait-per-instruction
    # hardware constraint that walrus codegen enforces.
    nc = bacc.Bacc("TRN2", target_bir_lowering=False, debug=False)
    x = nc.dram_tensor("x", [nb, ch, t], f32, kind="ExternalInput").ap()
    w = nc.dram_tensor("weights", [ch], f32, kind="ExternalInput").ap()
    y = nc.dram_tensor("y", [nb, ch, t], f32, kind="ExternalOutput").ap()

    with tile.TileContext(nc) as tc:
        with ExitStack() as ctx:
            cpool = ctx.enter_context(tc.tile_pool(name="const", bufs=1))
            xpool = ctx.enter_context(tc.tile_pool(name="xin", bufs=2))
            bpool = ctx.enter_context(tc.tile_pool(name="bout", bufs=3))

            # weights prep: w_clipped = clip(w, 0, 1); omw = 1 - w_clipped
            wt = cpool.tile([ch, 1], f32)
            nc.sync.dma_start(wt[:, 0:1], w.unsqueeze(1))
            wc = cpool.tile([ch, 1], f32)
            nc.vector.tensor_scalar(
                wc[:], wt[:], 0.0, 1.0, mybir.AluOpType.max, mybir.AluOpType.min
            )
            omw = cpool.tile([ch, 1], f32)
            nc.vector.tensor_scalar(
                omw[:], wc[:], -1.0, 1.0, mybir.AluOpType.mult, mybir.AluOpType.add
            )

            # reps>1 is a timing-only mode: repeat the identical computation
            # so one NEFF dispatch amortizes fixed overheads (see bench.py).
            for b in [i % nb for i in range(nb * reps)]:
                prev_tail = None
                for k in range(n_chunks):
                    sl = slice(k * t_chunk, (k + 1) * t_chunk)
                    X = xpool.tile([ch, t_chunk], f32)
                    nc.sync.dma_start(X[:], x[b][:, sl])
                    Bt = bpool.tile([ch, t_chunk], f32)
                    # b = w * x (per-partition scale). On DVE, not ACT: the
                    # ACT instruction encoding fits only one sync-wait, and
                    # this op needs two (DMA-in of X + pool-slot reuse).
                    nc.vector.tensor_scalar_mul(Bt[:], X[:], wc[:, 0:1])
                    # state = (1-w)*state + b along the free dim (in place)
                    init = X[:, 0:1] if k == 0 else prev_tail
                    nc.vector.tensor_tensor_scan(
                        Bt[:],
                        omw[:, 0:1].broadcast_to([ch, t_chunk]),
                        Bt[:],
                        init,
                        mybir.AluOpType.mult,
                        mybir.AluOpType.add,
                    )
                    nc.sync.dma_start(y[b][:, sl], Bt[:])
                    prev_tail = Bt[:, t_chunk - 1 : t_chunk]
    nc.compile()
    return nc


_nc_cache = None


def _get_nc():
    global _nc_cache
    if _nc_cache is None:
        _nc_cache = _build_bass()
    return _nc_cache


def _run(x, weights, trace=False):
    from concourse import bass_utils

    x = np.ascontiguousarray(np.asarray(x, dtype=np.float32))
    weights = np.ascontiguousarray(np.asarray(weights, dtype=np.float32))
    assert x.shape == (B, C, T), x.shape
    assert weights.shape == (C,), weights.shape

    nc = _get_nc()
    in_maps = [
        {"x": x[i * B_SHARD : (i + 1) * B_SHARD], "weights": weights}
        for i in range(N_CORES)
    ]
    res = bass_utils.run_bass_kernel_spmd(
        nc, in_maps, core_ids=list(range(N_CORES)), trace=trace
    )
    out = np.concatenate([r["y"] for r in res.results], axis=0)
    return out, res


def kernel(**inputs):
    out, _ = _run(inputs["x"], inputs["weights"])
    return out


# revision 13
# speedup vs baseline: 1.0203x; 1.0203x over previous
"""Trainium2 Bass kernel for per-channel EMA (first-order linear recurrence).

y[:, :, t] = w*x[:, :, t] + (1-w)*y[:, :, t-1],  y[:, :, -1] := x[:, :, 0]

Sharding: data-parallel over batch across 8 NeuronCores (8 batches/core).
Per core, per batch: channels (128) on the partition dim, time (8192) on the
free dim; premultiply b = w*x, then one native TensorTensorScan (both on the
Vector engine) computes the full recurrence. Memory-bound at the HBM
roofline: 64MB/core traffic at ~360GB/s ≈ 178us; measured ~170-200us.
"""

from contextlib import ExitStack

import numpy as np

# Hardcoded problem shape (self-contained; do not read spec/reference).
B, C, T = 64, 128, 8192
N_CORES = 8
B_SHARD = B // N_CORES


def _build_bass(nb=B_SHARD, ch=C, t=T, t_chunk=None, edge_chunk=None, reps=1):
    import concourse.tile as tile
    from concourse import bacc, mybir

    if t_chunk is None:
        t_chunk = t
    assert t % t_chunk == 0

    f32 = mybir.dt.float32
    # Bacc (not raw Bass): its compile() runs generate_event_semaphores(),
    # which splits multi-sem waits to satisfy the 1-wait-per-instruction
    # hardware constraint that walrus codegen enforces.
    nc = bacc.Bacc("TRN2", target_bir_lowering=False, debug=False)
    x = nc.dram_tensor("x", [nb, ch, t], f32, kind="ExternalInput").ap()
    w = nc.dram_tensor("weights", [ch], f32, kind="ExternalInput").ap()
    y = nc.dram_tensor("y", [nb, ch, t], f32, kind="ExternalOutput").ap()

    with tile.TileContext(nc) as tc:
        with ExitStack() as ctx:
            cpool = ctx.enter_context(tc.tile_pool(name="const", bufs=1))
            xpool = ctx.enter_context(tc.tile_pool(name="xin", bufs=2))
            bpool = ctx.enter_context(tc.tile_pool(name="bout", bufs=3))
            ipool = ctx.enter_context(tc.tile_pool(name="init", bufs=2))

            # weights prep: w_clipped = clip(w, 0, 1); omw = 1 - w_clipped
            wt = cpool.tile([ch, 1], f32)
            nc.sync.dma_start(wt[:, 0:1], w.unsqueeze(1))
            wc = cpool.tile([ch, 1], f32)
            nc.vector.tensor_scalar(
                wc[:], wt[:], 0.0, 1.0, mybir.AluOpType.max, mybir.AluOpType.min
            )
            omw = cpool.tile([ch, 1], f32)
            nc.vector.tensor_scalar(
                omw[:], wc[:], -1.0, 1.0, mybir.AluOpType.mult, mybir.AluOpType.add
            )

            # reps>1 is a timing-only mode: repeat the identical computation
            # so one NEFF dispatch amortizes fixed overheads (see bench.py).
            for i in range(nb * reps):
                b = i % nb
                # Optionally stream the first/last batch in smaller chunks:
                # the first batch gates pipeline fill, the last gates drain
                # (its Y-out can't start until its scan is done).
                tcb = t_chunk
                if edge_chunk is not None and (i == 0 or i == nb * reps - 1):
                    tcb = edge_chunk
                prev_tail = None
                for k in range(t // tcb):
                    sl = slice(k * tcb, (k + 1) * tcb)
                    X = xpool.tile([ch, tcb], f32, tag="X")
                    nc.sync.dma_start(X[:], x[b][:, sl])
                    if k == 0:
                        # Stage the scan's initial value (x[:,0]) in a tiny
                        # tile so the 4MB X tile's last reader is the premul,
                        # not the scan — frees X's pool slot ~8.5us earlier,
                        # closing a DMA idle gap (timeline analysis).
                        initc = ipool.tile([ch, 1], f32)
                        nc.vector.tensor_copy(initc[:], X[:, 0:1])
                    Bt = bpool.tile([ch, tcb], f32, tag="Bt")
                    # b = w * x (per-partition scale). On DVE, not ACT: the
                    # ACT instruction encoding fits only one sync-wait, and
                    # this op needs two (DMA-in of X + pool-slot reuse).
                    nc.vector.tensor_scalar_mul(Bt[:], X[:], wc[:, 0:1])
                    # state = (1-w)*state + b along the free dim (in place)
                    init = initc[:, 0:1] if k == 0 else prev_tail
                    nc.vector.tensor_tensor_scan(
                        Bt[:],
                        omw[:, 0:1].broadcast_to([ch, tcb]),
                        Bt[:],
                        init,
                        mybir.AluOpType.mult,
                        mybir.AluOpType.add,
                    )
                    nc.sync.dma_start(y[b][:, sl], Bt[:])
                    prev_tail = Bt[:, tcb - 1 : tcb]
    nc.compile()
    return nc


_nc_cache = None


def _get_nc():
    global _nc_cache
    if _nc_cache is None:
        # edge_chunk=2048: stream the first batch's fill and last batch's
        # drain in 1MB pieces — closes DMA idle at the stream edges
        # (timeline analysis: 197.8 -> 193.9us modeled).
        _nc_cache = _build_bass(edge_chunk=2048)
    return _nc_cache


def _run(x, weights, trace=False):
    from concourse import bass_utils

    x = np.ascontiguousarray(np.asarray(x, dtype=np.float32))
    weights = np.ascontiguousarray(np.asarray(weights, dtype=np.float32))
    assert x.shape == (B, C, T), x.shape
    assert weights.shape == (C,), weights.shape

    nc = _get_nc()
    in_maps = [
        {"x": x[i * B_SHARD : (i + 1) * B_SHARD], "weights": weights}
        for i in range(N_CORES)
    ]
    res = bass_utils.run_bass_kernel_spmd(
        nc, in_maps, core_ids=list(range(N_CORES)), trace=trace
    )
    out = np.concatenate([r["y"] for r in res.results], axis=0)
    return out, res


def kernel(**inputs):
    out, _ = _run(inputs["x"], inputs["weights"])
    return out
